# revision 11
# baseline (speedup 1.0000x reference)
"""Trainium2 Bass kernel for nn_MoEConnectionProcessor (v2: all-blockT).

Strategy
--------
Data-parallel over 8 cores (32768 cells each). Per core, cells are processed
in super-tiles (ST) of 2048 cells laid out "blockT": SBUF partition =
(g, d) with g = cell-subgroup (4 of 32 cells within a 128-cell tile),
d = feature; free axis = (t, c) = (tile-in-ST, cell-in-subgroup) = 512 cols.

The host pre-sorts each cell's 26 neighbors by tier and stages THREE
premasked copies of neighbor_states (tier-0/1/2 * nb), truncated to the
global max per-tier count W_t (~21), already in blockT with j (neighbor
slot) outermost. Because the masks are 0/1 and the tier classes partition
the neighbors:

  - S_t = sum_k m_t*nb   becomes an UNMASKED PE accumulation chain over j
    (identity stationary, premasked zeros contribute nothing) -> no DVE
    mask products, no transposes, no reduces.
  - tanh(m1 * msg) = m1 * tanh(msg) (b_msg == 0 per spec), so the
    functional expert's masked message sum is: matmul kron(I4, W_msg) per
    j-slot -> ACT tanh -> PE accumulation chain.
  - S0 = S_t0 + S_t1 + S_t2 (two cheap adds), loc_mean = S_t0/cnt0, etc.

All experts, gating, CNF steps and the final combine run in blockT
(biases are per-partition there). Gates ([12, 512] = (g, expert) rows)
are broadcast to 128 partitions with tiny scatter matmuls. Output stays
blockT in DRAM; the host inverse-permutes.

sigmoid(x) = 0.5*tanh(0.5x) + 0.5 and relu on DVE keep every ACT function
in one activation-table set (no ACT_TABLE_LOAD churn).
"""

import numpy as np
import ml_dtypes
from contextlib import ExitStack

import concourse.bass as bass
import concourse.bacc as bacc
import concourse.tile as tile
import concourse.mybir as mybir

B, K, D, NH = 262144, 26, 32, 32
N_CORES = 8
BS = B // N_CORES          # 32768 cells per core
ST = 2048                  # cells per super-tile
NT = BS // ST              # 16 super-tiles per core
TPS = ST // 128            # 16 tiles of 128 cells per super-tile
SC = TPS * 32              # 512 free columns per super-tile (t, c)
N_STEPS = 3
DT_STEP = 1.0 / N_STEPS

dt = mybir.dt
bf16 = ml_dtypes.bfloat16
f8e4 = ml_dtypes.float8_e4m3
AF = mybir.ActivationFunctionType
ALU = mybir.AluOpType

# staged dtype of the three big premasked neighbor copies
STAGE_DT = dt.bfloat16
STAGE_NP = bf16

# stationary slots in the packed weight tensor [128, n*128 + 12 + 4 + 3*128 + 12]
_WSLOTS = ["I128", "W4msg", "Wl_t", "Wl_b", "Wu_t", "Wu_b", "Wc_t", "Wc_b",
           "Wg1_t", "Wg1_b"]
# extra (non-128-wide) stationaries appended after the slots:
#   kron(I4, W_g2):      [128, 12]
#   ones_sum:            [12, 4]   (pad part-dim to 12 rows used)
#   recip bcast (f32):   [4, 12]
#   gate scatter e=0..2: [12, 128] each
EX_G2 = 128 * len(_WSLOTS)
EX_ONES = EX_G2 + 12
EX_SCAT = EX_ONES + 4          # 3x128 bf16 scatter
WC_COLS = EX_SCAT + 3 * 128
WF_COLS = 12                   # f32 tensor: recip-bcast [4, 12]
BC_COLS = 8                    # f32 biases


def _wslot(name):
    return 128 * _WSLOTS.index(name)


def build_program(w0, w1, w2):
    nc = bacc.Bacc("TRN2", target_bir_lowering=False, debug=False,
                   num_devices=N_CORES)

    a_m0 = nc.dram_tensor("m0", [64, NT * w0 * 2 * SC], dt.float8e4, kind="ExternalInput").ap()
    a_m1 = nc.dram_tensor("m1", [64, NT * w1 * 2 * SC], dt.float8e4, kind="ExternalInput").ap()
    a_m2 = nc.dram_tensor("m2", [64, NT * w2 * 2 * SC], dt.float8e4, kind="ExternalInput").ap()
    a_w8 = nc.dram_tensor("w8", [64, 512], dt.float8e4, kind="ExternalInput").ap()
    a_cst = nc.dram_tensor("cst", [128, NT * SC], dt.bfloat16, kind="ExternalInput").ap()
    a_icn = nc.dram_tensor("icn", [128, NT * 3 * SC], dt.bfloat16, kind="ExternalInput").ap()
    a_wc = nc.dram_tensor("wc", [128, WC_COLS], dt.bfloat16, kind="ExternalInput").ap()
    a_wf = nc.dram_tensor("wf", [4, WF_COLS], dt.float32, kind="ExternalInput").ap()
    a_bc = nc.dram_tensor("bc", [128, BC_COLS], dt.float32, kind="ExternalInput").ap()
    a_out = nc.dram_tensor("out", [128, NT * SC], dt.float32, kind="ExternalOutput").ap()

    with tile.TileContext(nc) as tc:
        _body(tc, a_m0, a_m1, a_m2, a_cst, a_icn, a_wc, a_wf, a_bc, a_w8,
              a_out, w0, w1, w2)
    nc.compile()
    return nc


def _body(tc, a_m0, a_m1, a_m2, a_cst, a_icn, a_wc, a_wf, a_bc, a_w8,
          a_out, w0, w1, w2):
    nc = tc.nc

    with ExitStack() as ctx:
        cpool = ctx.enter_context(tc.tile_pool(name="const", bufs=1))
        pin0 = ctx.enter_context(tc.tile_pool(name="in0", bufs=2))
        pin1 = ctx.enter_context(tc.tile_pool(name="in1", bufs=2))
        pin2 = ctx.enter_context(tc.tile_pool(name="in2", bufs=2))
        pinc = ctx.enter_context(tc.tile_pool(name="inc", bufs=2))
        ptnh = ctx.enter_context(tc.tile_pool(name="tnh", bufs=1))
        psml = ctx.enter_context(tc.tile_pool(name="sml", bufs=1))
        pout = ctx.enter_context(tc.tile_pool(name="out", bufs=2))
        # PSUM: chains 4 banks (bufs=1) + msgs 2 banks + experts 2 banks = 8
        pps_ch = ctx.enter_context(tc.tile_pool(name="pch", bufs=1, space="PSUM"))
        pps_m = ctx.enter_context(tc.tile_pool(name="pm", bufs=2, space="PSUM"))
        pps_e = ctx.enter_context(tc.tile_pool(name="pe", bufs=2, space="PSUM"))

        wc = cpool.tile([128, WC_COLS], dt.bfloat16, tag="wc")
        nc.sync.dma_start(wc[:], a_wc)
        wf = cpool.tile([4, WF_COLS], dt.float32, tag="wf")
        nc.sync.dma_start(wf[:], a_wf)
        bc = cpool.tile([128, BC_COLS], dt.float32, tag="bc")
        nc.sync.dma_start(bc[:], a_bc)
        w8 = cpool.tile([64, 512], dt.float8e4, tag="w8")
        nc.sync.dma_start(w8[:], a_w8)

        DR = mybir.MatmulPerfMode.DoubleRow

        def dr(ap2d):
            return ap2d.rearrange("p (two n) -> p two n", two=2)

        w8_I = dr(w8[:, 0:256])
        w8_msg = dr(w8[:, 256:512])

        def W(name):
            return wc[:, _wslot(name): _wslot(name) + 128]

        kron_g2 = wc[:, EX_G2:EX_G2 + 12]
        ones_sum = wc[0:12, EX_ONES:EX_ONES + 4]
        rb_f32 = wf[0:4, 0:12]
        scat = [wc[0:12, EX_SCAT + 128 * e: EX_SCAT + 128 * (e + 1)]
                for e in range(3)]

        b_loc4 = bc[:, 1:2]
        b_updh = bc[:, 2:3]   # 0.5 * b_upd, for sigmoid-via-tanh
        b_cnf4 = bc[:, 3:4]
        b_g14 = bc[:, 4:5]
        b_g2r = bc[0:12, 5:6]  # b_g2 on (g,e) rows 0..11
        b_msg4 = bc[:, 0:1]

        for i in range(NT):
            m0 = pin0.tile([64, w0 * 2 * SC], dt.float8e4, tag="m0")
            nc.sync.dma_start(m0[:], a_m0[:, i * w0 * 2 * SC:(i + 1) * w0 * 2 * SC])
            m1 = pin1.tile([64, w1 * 2 * SC], dt.float8e4, tag="m1")
            nc.sync.dma_start(m1[:], a_m1[:, i * w1 * 2 * SC:(i + 1) * w1 * 2 * SC])
            m2 = pin2.tile([64, w2 * 2 * SC], dt.float8e4, tag="m2")
            nc.sync.dma_start(m2[:], a_m2[:, i * w2 * 2 * SC:(i + 1) * w2 * 2 * SC])
            cst = pinc.tile([128, SC], dt.bfloat16, tag="cst")
            nc.sync.dma_start(cst[:], a_cst[:, i * SC:(i + 1) * SC])
            icn = pinc.tile([128, 3 * SC], dt.bfloat16, tag="icn")
            nc.sync.dma_start(icn[:], a_icn[:, i * 3 * SC:(i + 1) * 3 * SC])
            inv0 = icn[:, 0:SC]
            inv1 = icn[:, SC:2 * SC]
            inv2 = icn[:, 2 * SC:3 * SC]

            # ---- chain psums: St0 | St1 | St2 | agg (4 banks) ----
            pch = pps_ch.tile([128, 4 * SC], dt.float32, tag="ch")
            pSt0 = pch[:, 0:SC]
            pSt1 = pch[:, SC:2 * SC]
            pSt2 = pch[:, 2 * SC:3 * SC]
            pAgg = pch[:, 3 * SC:4 * SC]

            # raw tier-sum accumulation chains (fp8 DoubleRow, identity)
            for j in range(w0):
                nc.tensor.matmul(pSt0, w8_I, dr(m0[:, j * 2 * SC:(j + 1) * 2 * SC]),
                                 start=(j == 0), stop=(j == w0 - 1), perf_mode=DR)
            for j in range(w1):
                nc.tensor.matmul(pSt1, w8_I, dr(m1[:, j * 2 * SC:(j + 1) * 2 * SC]),
                                 start=(j == 0), stop=(j == w1 - 1), perf_mode=DR)
            for j in range(w2):
                nc.tensor.matmul(pSt2, w8_I, dr(m2[:, j * 2 * SC:(j + 1) * 2 * SC]),
                                 start=(j == 0), stop=(j == w2 - 1), perf_mode=DR)

            # ---- msgs: per-j matmul + tanh into SBUF, then accum chain ----
            tnh = ptnh.tile([128, w1 * SC], dt.bfloat16, tag="tnh")
            for j in range(w1):
                pm = pps_m.tile([128, SC], dt.float32, tag=f"pm")
                nc.tensor.matmul(pm[:], w8_msg, dr(m1[:, j * 2 * SC:(j + 1) * 2 * SC]),
                                 start=True, stop=True, perf_mode=DR)
                nc.scalar.activation(tnh[:, j * SC:(j + 1) * SC], pm[:],
                                     AF.Tanh, bias=b_msg4, scale=1.0)
            for j in range(w1):
                nc.tensor.matmul(pAgg, W("I128"), tnh[:, j * SC:(j + 1) * SC],
                                 start=(j == 0), stop=(j == w1 - 1))

            # ---- means / S0 (blockT, bf16 operands for expert matmuls) ----
            mloc = psml.tile([128, SC], dt.bfloat16, tag="mloc")
            nc.vector.tensor_tensor(out=mloc[:], in0=pSt0, in1=inv0, op=ALU.mult)
            mdis = psml.tile([128, SC], dt.bfloat16, tag="mdis")
            nc.vector.tensor_tensor(out=mdis[:], in0=pSt2, in1=inv2, op=ALU.mult)
            aggb = psml.tile([128, SC], dt.bfloat16, tag="aggb")
            nc.vector.tensor_tensor(out=aggb[:], in0=pAgg, in1=inv1, op=ALU.mult)
            st1c = psml.tile([128, SC], dt.bfloat16, tag="st1c")
            nc.scalar.copy(st1c[:], pSt1)
            s01 = psml.tile([128, SC], dt.bfloat16, tag="s01")
            nc.vector.tensor_tensor(out=s01[:], in0=pSt0, in1=st1c[:], op=ALU.add)
            s0 = psml.tile([128, SC], dt.bfloat16, tag="s0")
            nc.vector.tensor_tensor(out=s0[:], in0=pSt2, in1=s01[:], op=ALU.add)

            # ---- local expert: tanh([cs, loc_mean] @ W_local + b) ----
            pl = pps_e.tile([128, SC], dt.float32, tag="pe")
            nc.tensor.matmul(pl[:], W("Wl_t"), cst[:], start=True, stop=False)
            nc.tensor.matmul(pl[:], W("Wl_b"), mloc[:], start=False, stop=True)
            locb = psml.tile([128, SC], dt.bfloat16, tag="locb")
            nc.scalar.activation(locb[:], pl[:], AF.Tanh, bias=b_loc4, scale=1.0)

            # ---- func expert: z = sigmoid(u) = 0.5*tanh(0.5u + 0.5b) + 0.5
            pu = pps_e.tile([128, SC], dt.float32, tag="pe")
            nc.tensor.matmul(pu[:], W("Wu_t"), cst[:], start=True, stop=False)
            nc.tensor.matmul(pu[:], W("Wu_b"), aggb[:], start=False, stop=True)
            tu = psml.tile([128, SC], dt.bfloat16, tag="tu")
            nc.scalar.activation(tu[:], pu[:], AF.Tanh, bias=b_updh, scale=0.5)
            tagg = psml.tile([128, SC], dt.bfloat16, tag="tagg")
            nc.scalar.activation(tagg[:], aggb[:], AF.Tanh)
            d2 = psml.tile([128, SC], dt.bfloat16, tag="d2")
            nc.vector.tensor_tensor(out=d2[:], in0=tagg[:], in1=cst[:], op=ALU.subtract)
            e1 = psml.tile([128, SC], dt.bfloat16, tag="e1")
            nc.vector.scalar_tensor_tensor(out=e1[:], in0=tu[:], scalar=0.5,
                                           in1=d2[:], op0=ALU.mult, op1=ALU.mult)
            e2 = psml.tile([128, SC], dt.bfloat16, tag="e2")
            nc.vector.scalar_tensor_tensor(out=e2[:], in0=d2[:], scalar=0.5,
                                           in1=cst[:], op0=ALU.mult, op1=ALU.add)
            funcb = psml.tile([128, SC], dt.bfloat16, tag="funcb")
            nc.vector.tensor_tensor(out=funcb[:], in0=e1[:], in1=e2[:], op=ALU.add)

            # ---- distant expert: 3 Euler steps, x kept f32 ----
            xmm = cst            # bf16 matmul operand
            xf = None            # f32 state (None on step 0 => use cst)
            for s in range(N_STEPS):
                pc = pps_e.tile([128, SC], dt.float32, tag="pe")
                nc.tensor.matmul(pc[:], W("Wc_t"), xmm[:], start=True, stop=False)
                nc.tensor.matmul(pc[:], W("Wc_b"), mdis[:], start=False, stop=True)
                vb = psml.tile([128, SC], dt.float32, tag=f"vb{s}")
                nc.scalar.activation(vb[:], pc[:], AF.Tanh, bias=b_cnf4, scale=1.0)
                xn = psml.tile([128, SC], dt.float32, tag=f"xn{s}")
                nc.vector.scalar_tensor_tensor(out=xn[:], in0=vb[:], scalar=DT_STEP,
                                               in1=(xf[:] if xf is not None else cst[:]),
                                               op0=ALU.mult, op1=ALU.add)
                xf = xn
                if s < N_STEPS - 1:
                    xmm = psml.tile([128, SC], dt.bfloat16, tag=f"xm{s}")
                    nc.scalar.copy(xmm[:], xn[:])
            xb = xf

            # ---- gating ----
            pg = pps_e.tile([128, SC], dt.float32, tag="pe")
            nc.tensor.matmul(pg[:], W("Wg1_t"), cst[:], start=True, stop=False)
            nc.tensor.matmul(pg[:], W("Wg1_b"), s0[:], start=False, stop=True)
            hb = psml.tile([128, SC], dt.bfloat16, tag="hb")
            nc.vector.tensor_scalar(out=hb[:], in0=pg[:], scalar1=b_g14,
                                    scalar2=0.0, op0=ALU.add, op1=ALU.max)
            pl2 = pps_e.tile([128, SC], dt.float32, tag="pe")
            nc.tensor.matmul(pl2[0:12, :], kron_g2, hb[:], start=True, stop=True)
            eg = psml.tile([12, SC], dt.bfloat16, tag="eg")
            nc.scalar.activation(eg[:], pl2[0:12, :], AF.Exp, bias=b_g2r, scale=1.0)
            ps = pps_e.tile([128, SC], dt.float32, tag="pe")
            nc.tensor.matmul(ps[0:4, :], ones_sum, eg[:], start=True, stop=True)
            rec = psml.tile([4, SC], dt.float32, tag="rec")
            nc.vector.reciprocal_approx_fast(out=rec[:], in_=ps[0:4, :])
            prb = pps_e.tile([128, SC], dt.float32, tag="pe")
            nc.tensor.matmul(prb[0:12, :], rb_f32, rec[:], start=True, stop=True)
            gts = psml.tile([12, SC], dt.bfloat16, tag="gts")
            nc.vector.tensor_tensor(out=gts[:], in0=eg[:], in1=prb[0:12, :], op=ALU.mult)

            # gate broadcast (12 -> 128 partitions) + weighted combine.
            # ge psums come one at a time from the msgs pool so the chain
            # banks free up right after the means (next ST's chains start).
            exps = [locb, funcb, xb]
            accs = []
            for e in range(3):
                pge = pps_m.tile([128, SC], dt.float32, tag="pm")
                nc.tensor.matmul(pge[:], scat[e], gts[:], start=True, stop=True)
                ae = psml.tile([128, SC], dt.float32, tag=f"ae{e}")
                nc.vector.tensor_tensor(out=ae[:], in0=pge[:], in1=exps[e][:], op=ALU.mult)
                accs.append(ae)
            a1, a2, a3 = accs
            a12 = psml.tile([128, SC], dt.float32, tag="a12")
            nc.vector.tensor_tensor(out=a12[:], in0=a1[:], in1=a2[:], op=ALU.add)
            outb = pout.tile([128, SC], dt.float32, tag="outb")
            nc.vector.tensor_tensor(out=outb[:], in0=a12[:], in1=a3[:], op=ALU.add)

            nc.sync.dma_start(a_out[:, i * SC:(i + 1) * SC], outb[:])


# ---------------------------------------------------------------------------
# host staging
# ---------------------------------------------------------------------------

def _to_blockT(arr_bsd):
    """[bs, d] (d == 32) -> blockT [128, NT*SC]: partition = g*32+d,
    cols = (i, t, c)."""
    bs, d = arr_bsd.shape
    a = arr_bsd.reshape(NT, TPS, 4, 32, d)           # [i, t, g, c, d]
    a = a.transpose(2, 4, 0, 1, 3)                   # [g, d, i, t, c]
    return np.ascontiguousarray(a.reshape(128, NT * SC))


def _nb_blockT(nb_sel):
    """[bs, w, 32] premasked sorted neighbors -> [128, NT*w*SC]:
    partition = g*32+d, cols = (i, j, t, c)."""
    bs, w, d = nb_sel.shape
    a = nb_sel.reshape(NT, TPS, 4, 32, w, d)         # [i, t, g, c, j, d]
    a = a.transpose(2, 5, 0, 4, 1, 3)                # [g, d, i, j, t, c]
    return np.ascontiguousarray(a.reshape(128, NT * w * SC))


def _nb_blockT_dr(nb_sel):
    """DoubleRow fp8 packing: [128, NT*w*SC] -> [64, NT*w*2*SC] with
    partition row kt*64+p mapped to free slot (kt)."""
    bt = _nb_blockT(nb_sel)
    w = nb_sel.shape[1]
    a = bt.reshape(2, 64, NT, w, SC)                 # [kt, p, i, j, n]
    a = a.transpose(1, 2, 3, 0, 4)                   # [p, i, j, kt, n]
    return np.ascontiguousarray(a.reshape(64, NT * w * 2 * SC)).astype(f8e4)


def _dr_pack_w(mat128):
    """[128, M] stationary -> [64, 2*M] DoubleRow layout."""
    m = mat128.shape[1]
    a = mat128.reshape(2, 64, m).transpose(1, 0, 2)
    return np.ascontiguousarray(a.reshape(64, 2 * m))


def _from_blockT(arr):
    """inverse of _to_blockT: [128, NT*SC] -> [bs, 32]."""
    a = arr.reshape(4, 32, NT, TPS, 32)              # [g, d, i, t, c]
    a = a.transpose(2, 3, 0, 4, 1)                   # [i, t, g, c, d]
    return np.ascontiguousarray(a.reshape(NT * ST, 32))


def stage_weights(inputs, widths):
    f32 = np.float32
    W_local = np.asarray(inputs["W_local"], f32)
    W_msg = np.asarray(inputs["W_msg"], f32)
    W_upd = np.asarray(inputs["W_upd"], f32)
    W_cnf = np.asarray(inputs["W_cnf"], f32)
    W_g1 = np.asarray(inputs["W_g1"], f32)
    W_g2 = np.asarray(inputs["W_g2"], f32)

    eye4 = np.eye(4, dtype=f32)

    def kron4(w):
        return np.kron(eye4, w)

    wparts = {
        "I128": np.eye(128, dtype=f32),
        "W4msg": kron4(W_msg),
        "Wl_t": kron4(W_local[:D]), "Wl_b": kron4(W_local[D:]),
        "Wu_t": kron4(W_upd[:D]), "Wu_b": kron4(W_upd[D:]),
        "Wc_t": kron4(W_cnf[:D]), "Wc_b": kron4(W_cnf[D:]),
        "Wg1_t": kron4(W_g1[:D]), "Wg1_b": kron4(W_g1[D:] / K),
    }
    wc = np.zeros((128, WC_COLS), f32)
    for name in _WSLOTS:
        wc[:, _wslot(name):_wslot(name) + 128] = wparts[name]
    # kron(I4, W_g2): [128, 12]
    for g in range(4):
        wc[32 * g:32 * (g + 1), EX_G2 + 3 * g:EX_G2 + 3 * (g + 1)] = W_g2
    # ones_sum [12, 4]: row (g,e) -> col g
    for g in range(4):
        for e in range(3):
            wc[3 * g + e, EX_ONES + g] = 1.0
    # gate scatter: e fixed: [12, 128]: row (g,e') -> cols (g, d) if e'==e
    for e in range(3):
        for g in range(4):
            wc[3 * g + e, EX_SCAT + 128 * e + 32 * g:
               EX_SCAT + 128 * e + 32 * (g + 1)] = 1.0
    wc = wc.astype(bf16)

    wf = np.zeros((4, WF_COLS), f32)
    for g in range(4):
        wf[g, 3 * g:3 * (g + 1)] = 1.0   # recip bcast [4, 12]

    w8 = np.zeros((64, 512), f32)
    w8[:, 0:256] = _dr_pack_w(np.eye(128, dtype=f32))
    w8[:, 256:512] = _dr_pack_w(wparts["W4msg"])
    w8 = w8.astype(f8e4)

    bcq = np.zeros((128, BC_COLS), f32)
    bcq[:, 0] = np.tile(np.asarray(inputs["b_msg"], f32), 4)
    bcq[:, 1] = np.tile(np.asarray(inputs["b_local"], f32), 4)
    bcq[:, 2] = 0.5 * np.tile(np.asarray(inputs["b_upd"], f32), 4)
    bcq[:, 3] = np.tile(np.asarray(inputs["b_cnf"], f32), 4)
    bcq[:, 4] = np.tile(np.asarray(inputs["b_g1"], f32), 4)
    b_g2 = np.asarray(inputs["b_g2"], f32)
    for g in range(4):
        bcq[3 * g:3 * (g + 1), 5] = b_g2
    return wc, wf, bcq, w8


def stage_inputs(inputs):
    """Returns (in_maps, widths)."""
    f32 = np.float32
    cs = np.asarray(inputs["current_state"], f32)
    nb = np.asarray(inputs["neighbor_states"], f32)
    tiers = np.asarray(inputs["tier_ids"], np.int32)

    if np.any(np.asarray(inputs["b_msg"], f32) != 0.0):
        raise NotImplementedError("premask trick requires b_msg == 0")

    cnt = np.stack([(tiers == t).sum(-1) for t in range(3)], axis=1).astype(f32)  # [B, 3]
    widths = tuple(int(cnt[:, t].max()) for t in range(3))

    # per-tier sorted+premasked neighbor copies, truncated to widths
    copies = []
    for t in range(3):
        order = np.argsort(tiers != t, axis=1, kind="stable")[:, :widths[t]]
        sel = np.take_along_axis(nb, order[:, :, None], axis=1)
        msk = np.take_along_axis(tiers == t, order, axis=1)
        copies.append((sel * msk[:, :, None]).astype(np.float32))

    inv = 1.0 / np.maximum(cnt, 1.0)       # [B, 3]

    wc, wf, bcq, w8 = stage_weights(inputs, widths)

    in_maps = []
    for c in range(N_CORES):
        rs = slice(c * BS, (c + 1) * BS)
        icn = np.empty((128, NT * 3 * SC), bf16)
        iv = [_to_blockT(np.repeat(inv[rs, t:t + 1], D, axis=1)) for t in range(3)]
        for i in range(NT):
            for t in range(3):
                icn[:, (3 * i + t) * SC:(3 * i + t + 1) * SC] = \
                    iv[t][:, i * SC:(i + 1) * SC]
        in_maps.append({
            "m0": _nb_blockT_dr(copies[0][rs]),
            "m1": _nb_blockT_dr(copies[1][rs]),
            "m2": _nb_blockT_dr(copies[2][rs]),
            "cst": _to_blockT(cs[rs]).astype(bf16),
            "icn": icn,
            "wc": wc, "wf": wf, "bc": bcq, "w8": w8,
        })
    return in_maps, widths


_PROGRAM_CACHE = {}


def kernel(**inputs):
    from concourse.bass_utils import run_bass_kernel_spmd

    in_maps, widths = stage_inputs(inputs)
    if widths not in _PROGRAM_CACHE:
        _PROGRAM_CACHE[widths] = build_program(*widths)
    nc = _PROGRAM_CACHE[widths]

    res = run_bass_kernel_spmd(nc, in_maps, core_ids=list(range(N_CORES)))
    out = np.concatenate([_from_blockT(np.asarray(r["out"], np.float32))
                          for r in res.results], axis=0)
    return out.astype(np.float32)
